# revision 1
# baseline (speedup 1.0000x reference)
"""CorefHead Trainium2 kernel.

Reference computation (B=64, S=512, H=1024, HID=512):
  emb_a = span_mean(bert, offsets[:,0:2])   # [B,H]
  emb_b = span_mean(bert, offsets[:,2:4])   # [B,H]
  emb_p = bert[b, offsets[:,4]]             # [B,H]
  x = concat([emb_a, emb_b, emb_p], -1)     # [B,3H]
  h = leaky_relu(batchnorm_eval(x @ W1 + b1), 0.01)
  out = h @ W2 + b2                         # [B,3]

Strategy: pure data parallel, batch sharded 8 ways (8 batches/core).
Per core:
  - Host precomputes prescaled span masks (1/len) + pron one-hot, packed
    [128, nch, 3]; host also packs only the needed row-window of bert per
    batch (union of span/pron rows, padded to 128-row chunks).
  - mm1 (PE): xT[h_chunk] += bert_chunk.T @ mask_chunk -> x transposed
    [3072, 8] directly (no on-device transpose needed).
  - mm2 (PE): h[8, 512] += xT_chunk.T @ W1_chunk over 24 K-chunks.
  - BN+LeakyReLU (DVE): y = max(t, 0.01*t), t = h*scale + bias with
    scale/bias folded from (b1, gamma, beta, running stats) on host.
  - mm3 (DVE): out[:, j] = b2[j] + sum(y * W2[:, j]) via tensor_tensor_reduce.
Host gathers per-core [8, 3] outputs and undoes the batch permutation.
"""

import numpy as np

B, S, H = 64, 512, 1024
HID = 512
EPS = 1e-5
NCORES = 8
BPC = B // NCORES  # batches per core
KC = 3 * H // 128  # 24 contraction chunks for mm2
HC = H // 128      # 8 h-chunks per embedding

# Set to True to ship only the needed row-window of bert per batch.
WINDOW = True
# bert/masks in bf16: halves the dominant DMA traffic and runs mm1 single
# pass with fast weight load on the PE. Masks hold exact 0/1 (bf16-exact);
# the 1/span_len scaling happens in fp32 on the PSUM->SBUF copy.
BERT_BF16 = True
# W1 (and the mm2 x operand) in bf16.
W1_BF16 = True
# Ship pron rows fp32 + transpose on device. Only buys precision when the
# mm2 operands stay fp32 (otherwise x is rounded to bf16 anyway).
PRON_FP32 = not W1_BF16

# Test-harness hooks (harness calls kernel() with TRACE=False default).
TRACE = False
LAST_RESULT = None

_PROGRAM_CACHE: dict = {}


def _build_program(nch_slots: tuple):
    """Build + compile the SPMD Bass program for the given per-slot chunk
    counts (number of 128-row S-chunks shipped per batch slot)."""
    import concourse.bacc as bacc
    import concourse.tile as tile
    import concourse.mybir as mybir
    from concourse.bass import MemorySpace

    f32 = mybir.dt.float32
    bdt = mybir.dt.bfloat16 if BERT_BF16 else f32
    wdt = mybir.dt.bfloat16 if W1_BF16 else f32
    ncht = int(sum(nch_slots))
    totrows = ncht * 128

    nc = bacc.Bacc("TRN2", target_bir_lowering=False, debug=False,
                   num_devices=NCORES)

    nmc = 2 if PRON_FP32 else 3  # mask columns (spans only, or spans+pron)

    bert_d = nc.dram_tensor("bertw", [totrows, H], bdt, kind="ExternalInput").ap()
    mask_d = nc.dram_tensor("maskp", [128, ncht, nmc], bdt, kind="ExternalInput").ap()
    sfac_d = nc.dram_tensor("sfac", [128, BPC, nmc], f32, kind="ExternalInput").ap()
    if PRON_FP32:
        pron_d = nc.dram_tensor("pron", [BPC, H], f32, kind="ExternalInput").ap()
    w1_d = nc.dram_tensor("w1", [3 * H, HID], wdt, kind="ExternalInput").ap()
    # bnbP[p, mc] = folded BN bias for hid index mc*128+p; w2P[p, mc, j] =
    # W2[mc*128+p, j]; b2c = b2[:, None]
    bnb_d = nc.dram_tensor("bnbP", [128, HID // 128], f32,
                           kind="ExternalInput").ap()
    w2_d = nc.dram_tensor("w2P", [128, HID // 128, 3], f32,
                          kind="ExternalInput").ap()
    b2_d = nc.dram_tensor("b2c", [3, 1], f32, kind="ExternalInput").ap()
    out_d = nc.dram_tensor("out", [3, BPC], f32, kind="ExternalOutput").ap()

    chbase = np.concatenate([[0], np.cumsum(nch_slots)]).astype(int)

    with tile.TileContext(nc) as tc:
        with (
            tc.tile_pool(name="singles", bufs=1) as singles,
            tc.tile_pool(name="bert_pool", bufs=3) as bert_pool,
            tc.tile_pool(name="head", bufs=1) as head,
            tc.tile_pool(name="psum_x", bufs=4, space=MemorySpace.PSUM) as psum_x_pool,
            tc.tile_pool(name="psum_p", bufs=2, space=MemorySpace.PSUM) as psum_p_pool,
            tc.tile_pool(name="psum_h", bufs=1, space=MemorySpace.PSUM) as psum_h_pool,
        ):
            # --- constant / parameter loads ---
            mask_t = singles.tile([128, ncht, nmc], bdt)
            nc.scalar.dma_start(out=mask_t, in_=mask_d)
            sfac_t = singles.tile([128, BPC, nmc], f32)
            nc.scalar.dma_start(out=sfac_t, in_=sfac_d)
            if PRON_FP32:
                pron_t = singles.tile([BPC, H], f32)
                nc.scalar.dma_start(out=pron_t, in_=pron_d)
            bnb_t = head.tile([128, HID // 128], f32)
            nc.scalar.dma_start(out=bnb_t, in_=bnb_d)
            w2_t = head.tile([128, HID // 128, 3], f32)
            nc.scalar.dma_start(out=w2_t, in_=w2_d)
            b2_t = head.tile([3, 1], f32)
            nc.scalar.dma_start(out=b2_t, in_=b2_d)
            from concourse.masks import make_identity
            idt = singles.tile([BPC, BPC], f32)
            make_identity(nc, idt)

            # xT accumulator: [128, 3 embeddings, HC chunks * BPC batches]
            # column for contraction-chunk kc=(e*HC+hc), batch b is
            # xT[:, e, hc*BPC + b]  -> mm2 rhs slice [128, BPC] contiguous.
            xT_t = singles.tile([128, 3, HC * BPC], wdt)

            if PRON_FP32:
                # --- pron embedding: exact fp32 rows, transposed via PE ---
                for hc in range(HC):
                    pxp = psum_p_pool.tile([128, BPC], f32, tag="pxp")
                    nc.tensor.transpose(
                        pxp, pron_t[:, hc * 128:(hc + 1) * 128], idt)
                    nc.vector.tensor_copy(
                        xT_t[:, 2, hc * BPC:(hc + 1) * BPC], pxp)

            # --- mm1: per-batch span sums (bert chunks as PE weights) ---
            # W1 + consts ride the ACT HWDGE ring; berts ride the SP ring.
            # The SDMA engines round-robin between the rings, so the bert
            # stream is never stuck behind the big W1 transfer.
            w1_t = singles.tile([128, KC, HID], wdt)
            w1_src = w1_d.rearrange("(kc p) n -> p kc n", p=128)
            for i in range(4):
                nc.scalar.dma_start(
                    out=w1_t[:, 6 * i:6 * (i + 1), :],
                    in_=w1_src[:, 6 * i:6 * (i + 1), :])
            # bert slots DMA'd in pairs (~1.3-2.1 MB per transfer) for
            # better SDMA efficiency while still overlapping mm1.
            for pair in range(BPC // 2):
                b0 = 2 * pair
                nchp = int(nch_slots[b0]) + int(nch_slots[b0 + 1])
                bt = bert_pool.tile([128, nchp, H], bdt, tag="bert")
                r0 = int(chbase[b0]) * 128
                nc.sync.dma_start(
                    out=bt[:, :nchp, :],
                    in_=bert_d[r0:r0 + nchp * 128, :].rearrange(
                        "(sc p) h -> p sc h", p=128))
                for b in (b0, b0 + 1):
                    nch = int(nch_slots[b])
                    sc0 = int(chbase[b]) - int(chbase[b0])
                    for hc in range(HC):
                        px = psum_x_pool.tile([128, nmc], f32)
                        for sc in range(nch):
                            nc.tensor.matmul(
                                px,
                                bt[:, sc0 + sc, hc * 128:(hc + 1) * 128],
                                mask_t[:, int(chbase[b]) + sc, :],
                                start=(sc == 0),
                                stop=(sc == nch - 1),
                            )
                        # fp32 scale by (1/lenA, 1/lenB[, 1]), PSUM->SBUF
                        nc.vector.tensor_mul(
                            xT_t[:, 0:nmc, hc * BPC + b], px, sfac_t[:, b, :])

            # --- mm2: h[BPC, HID] = x @ (W1 * bn_scale) over 24 K-chunks ---
            # (the BN eval-mode scale is folded into W1 on the host)
            ph = psum_h_pool.tile([BPC, HID], f32)
            for kc in range(KC):
                e, hc = kc // HC, kc % HC
                nc.tensor.matmul(
                    ph,
                    xT_t[:, e, hc * BPC:(hc + 1) * BPC],
                    w1_t[:, kc, :],
                    start=(kc == 0),
                    stop=(kc == KC - 1),
                )
            hs_t = head.tile([BPC, HID], f32)
            nc.vector.tensor_copy(hs_t, ph)

            # --- per hid-chunk: transpose h, + BN bias, LeakyReLU, mm3 ---
            ot_ps = psum_h_pool.tile([3, BPC], f32, tag="oT")
            for mc in range(HID // 128):
                pht = psum_p_pool.tile([128, BPC], f32, tag="pht")
                nc.tensor.transpose(
                    pht, hs_t[:, mc * 128:(mc + 1) * 128], idt)
                t_t = head.tile([128, BPC], f32, tag="t_t")
                nc.vector.tensor_scalar_add(t_t, pht, bnb_t[:, mc:mc + 1])
                y_t = head.tile([128, BPC], f32, tag="y_t")
                # y = max(0.01 * t, t)
                nc.vector.scalar_tensor_tensor(
                    y_t, t_t, 0.01, t_t,
                    op0=mybir.AluOpType.mult, op1=mybir.AluOpType.max)
                nc.tensor.matmul(
                    ot_ps, w2_t[:, mc, :], y_t,
                    start=(mc == 0), stop=(mc == HID // 128 - 1))

            o_t = head.tile([3, BPC], f32)
            nc.vector.tensor_scalar_add(o_t, ot_ps, b2_t)
            nc.sync.dma_start(out=out_d, in_=o_t)

    nc.compile()
    return nc


def _prep_core_inputs(bert, bert_f32, offsets, w1, bnbP, w2P, b2c,
                      batch_idx, nch_slots):
    """Build the per-core input map for the given batch indices."""
    nmc = 2 if PRON_FP32 else 3
    ncht = int(sum(nch_slots))
    bertw = np.empty((ncht * 128, H), dtype=bert.dtype)
    maskp = np.zeros((128, ncht, nmc), dtype=bert.dtype)
    sfac = np.ones((BPC, nmc), dtype=np.float32)
    pron = np.empty((BPC, H), dtype=np.float32)
    row = 0
    for slot, gb in enumerate(batch_idx):
        nch = int(nch_slots[slot])
        L = nch * 128
        a0, a1, b0, b1_, p = (int(v) for v in offsets[gb])
        lo = min(a0, b0, p)
        w0 = max(0, min(lo, S - L))
        bertw[row:row + L] = bert[gb, w0:w0 + L]
        pron[slot] = bert_f32[gb, p]
        pos = w0 + np.arange(L)
        cols = [((pos >= a0) & (pos <= a1)).astype(np.float32),
                ((pos >= b0) & (pos <= b1_)).astype(np.float32)]
        if not PRON_FP32:
            cols.append((pos == p).astype(np.float32))
        sfac[slot, 0] = 1.0 / (a1 - a0 + 1)
        sfac[slot, 1] = 1.0 / (b1_ - b0 + 1)
        blk = np.stack(cols, axis=-1).reshape(nch, 128, nmc)
        maskp[:, row // 128:row // 128 + nch, :] = blk.transpose(1, 0, 2)
        row += L
    in_map = {
        "bertw": bertw,
        "maskp": maskp,
        "sfac": np.broadcast_to(sfac, (128, BPC, nmc)).copy(),
        "w1": w1,
        "bnbP": bnbP,
        "w2P": w2P,
        "b2c": b2c,
    }
    if PRON_FP32:
        in_map["pron"] = pron
    return in_map


def kernel(bert_outputs, offsets, W1, b1, gamma, beta, running_mean,
           running_var, W2, b2):
    import ml_dtypes

    bert_f32 = np.ascontiguousarray(np.asarray(bert_outputs, dtype=np.float32))
    bert = bert_f32.astype(ml_dtypes.bfloat16) if BERT_BF16 else bert_f32
    offs = np.asarray(offsets).astype(np.int64)
    W1 = np.asarray(W1, dtype=np.float32)
    b1 = np.asarray(b1, dtype=np.float32)
    gamma = np.asarray(gamma, dtype=np.float32)
    beta = np.asarray(beta, dtype=np.float32)
    rm = np.asarray(running_mean, dtype=np.float32)
    rv = np.asarray(running_var, dtype=np.float32)
    W2 = np.asarray(W2, dtype=np.float32)
    b2 = np.asarray(b2, dtype=np.float32)

    # Fold BN eval-mode stats: bn(xW1 + b1) = x(W1*s) + ((b1 - mean)*s + beta)
    s = gamma / np.sqrt(rv + EPS)
    bias = (b1 - rm) * s + beta
    W1 = np.ascontiguousarray(W1 * s[None, :])
    if W1_BF16:
        W1 = W1.astype(ml_dtypes.bfloat16)
    bnbP = np.ascontiguousarray(bias.reshape(HID // 128, 128).T)
    w2P = np.ascontiguousarray(
        W2.reshape(HID // 128, 128, 3).transpose(1, 0, 2))
    b2c = np.ascontiguousarray(b2.reshape(3, 1))

    # Row windows: union of span/pron rows per batch, padded to 128-row
    # chunks. Sort batches by window size so same-slot batches across cores
    # share one (max) chunk count; undone at gather time.
    if WINDOW:
        lo = offs[:, [0, 2, 4]].min(axis=1)
        hi = offs[:, [1, 3, 4]].max(axis=1)
        lens = np.minimum((hi - lo + 128) // 128 * 128, S)
        order = np.argsort(-lens, kind="stable")
    else:
        lens = np.full(B, S, dtype=np.int64)
        order = np.arange(B)

    # slot i of every core holds batches ranked [i*NCORES, (i+1)*NCORES)
    perm = order.reshape(BPC, NCORES)  # [slot, core] -> global batch
    nch_slots = tuple(int(lens[perm[i]].max()) // 128 for i in range(BPC))

    key = nch_slots
    if key not in _PROGRAM_CACHE:
        _PROGRAM_CACHE[key] = _build_program(key)
    nc = _PROGRAM_CACHE[key]

    in_maps = [
        _prep_core_inputs(bert, bert_f32, offs, W1, bnbP, w2P, b2c,
                          perm[:, c], nch_slots)
        for c in range(NCORES)
    ]

    from concourse import bass_utils
    kwargs = {}
    if TRACE:
        kwargs = {"trace": True, "trace_cores": list(range(NCORES))}
    res = bass_utils.run_bass_kernel_spmd(nc, in_maps,
                                          core_ids=list(range(NCORES)),
                                          **kwargs)
    global LAST_RESULT
    LAST_RESULT = res

    out = np.empty((B, 3), dtype=np.float32)
    for c in range(NCORES):
        out[perm[:, c]] = res.results[c]["out"].T
    return out



# revision 7
# speedup vs baseline: 1.2795x; 1.2795x over previous
"""CorefHead Trainium2 kernel.

Reference computation (B=64, S=512, H=1024, HID=512):
  emb_a = span_mean(bert, offsets[:,0:2])   # [B,H]
  emb_b = span_mean(bert, offsets[:,2:4])   # [B,H]
  emb_p = bert[b, offsets[:,4]]             # [B,H]
  x = concat([emb_a, emb_b, emb_p], -1)     # [B,3H]
  h = leaky_relu(batchnorm_eval(x @ W1 + b1), 0.01)
  out = h @ W2 + b2                         # [B,3]

Strategy: pure data parallel, batch sharded 8 ways (8 batches/core),
DMA-volume minimized:
  - Host ships only the exact union rows (span A + span B) per batch,
    packed back-to-back across the core's 8 batches into 128-row chunks
    (chunks may cross batch boundaries). Rows are fp8-e4m3: span means
    average ~170 rows and the pron row dominates the final signal, so
    fp8 noise on span rows stays ~0.5% at the output. The pron rows ship
    separately in fp32 and are transposed on the PE.
  - mm1 (PE): per h-chunk a single PSUM tile [128, 16] accumulates
    bert_chunk.T @ mask_chunk over ALL row chunks; mask has one column
    per (span, slot) so batch identity lives in the mask column.
  - mm2 (PE, swapped operands): phT[q] += W1sub[128k, 128hid].T @
    xT[128k, 8] over 24 k-chunks -> h transposed [512, 8] directly (no
    on-device transpose of h), streaming only 8 columns per matmul.
  - BN+LeakyReLU (DVE) on hT tiles; mm3 (PE): out[3, 8] += w2q.T @ yq.
  - DMA: bert rides the SP ring in ~0.5 MB groups (first group small to
    prime the mm1 pipeline); consts + W1 ride the ACT ring; W1 is only
    needed by mm2 at the end so bert is never stuck behind it.
Host gathers per-core [3, 8] outputs and undoes the batch permutation.
"""

import numpy as np

B, S, H = 64, 512, 1024
HID = 512
EPS = 1e-5
NCORES = 8
BPC = B // NCORES  # batches per core
KC = 3 * H // 128  # 24 contraction chunks for mm2
HC = H // 128      # 8 h-chunks per embedding
NQ = HID // 128    # 4 hid quarters

# bert span rows + masks in fp8-e4m3 (halves DMA vs bf16); pron fp32.
BERT_FP8 = False
# W1 (and the mm2 xT operand) in bf16.
W1_BF16 = True

# Test-harness hooks (harness calls kernel() with TRACE=False default).
TRACE = False
LAST_RESULT = None

_PROGRAM_CACHE: dict = {}


def _bert_groups(totch: int):
    """Chunk-group sizes for the bert DMA: small first group to prime
    the mm1 pipeline, then ~4-chunk (512 KB fp8) transfers."""
    if totch <= 2:
        return [totch]
    groups = [2]
    left = totch - 2
    while left > 0:
        g = min(4, left)
        groups.append(g)
        left -= g
    return groups


def _build_program(totch: int):
    import concourse.bacc as bacc
    import concourse.tile as tile
    import concourse.mybir as mybir
    from concourse.bass import MemorySpace
    from concourse.masks import make_identity

    f32 = mybir.dt.float32
    bdt = mybir.dt.float8e4 if BERT_FP8 else mybir.dt.bfloat16
    wdt = mybir.dt.bfloat16 if W1_BF16 else f32

    nc = bacc.Bacc("TRN2", target_bir_lowering=False, debug=False,
                   num_devices=NCORES)

    NMC = 2 * BPC  # mask columns: (span e, slot) -> e*BPC + slot

    bert_d = nc.dram_tensor("bertp", [128, totch * H], bdt,
                            kind="ExternalInput").ap()
    mask_d = nc.dram_tensor("maskp", [128, totch, NMC], bdt,
                            kind="ExternalInput").ap()
    sfac_d = nc.dram_tensor("sfac", [128, NMC], f32, kind="ExternalInput").ap()
    pron_d = nc.dram_tensor("pron", [BPC, H], f32, kind="ExternalInput").ap()
    w1_d = nc.dram_tensor("w1P", [128, KC, HID], wdt, kind="ExternalInput").ap()
    bnb_d = nc.dram_tensor("bnbP", [128, NQ], f32, kind="ExternalInput").ap()
    w2_d = nc.dram_tensor("w2P", [128, NQ, 3], f32, kind="ExternalInput").ap()
    b2_d = nc.dram_tensor("b2c", [3, 1], f32, kind="ExternalInput").ap()
    out_d = nc.dram_tensor("out", [3, BPC], f32, kind="ExternalOutput").ap()

    groups = _bert_groups(totch)

    with tile.TileContext(nc) as tc:
        with (
            tc.tile_pool(name="singles", bufs=1) as singles,
            tc.tile_pool(name="head", bufs=1) as head,
            tc.tile_pool(name="psum_x", bufs=3, space=MemorySpace.PSUM) as psum_x,
            tc.tile_pool(name="psum_p", bufs=2, space=MemorySpace.PSUM) as psum_p,
            tc.tile_pool(name="psum_h", bufs=1, space=MemorySpace.PSUM) as psum_h,
        ):
            # --- consts on the ACT ring (tiny, land first) ---
            mask_t = singles.tile([128, totch, NMC], bdt)
            nc.scalar.dma_start(out=mask_t, in_=mask_d)
            sfac_t = singles.tile([128, NMC], f32)
            nc.scalar.dma_start(out=sfac_t, in_=sfac_d)
            pron_t = singles.tile([BPC, H], f32)
            nc.scalar.dma_start(out=pron_t, in_=pron_d)
            bnb_t = head.tile([128, NQ], f32)
            nc.scalar.dma_start(out=bnb_t, in_=bnb_d)
            w2_t = head.tile([128, NQ, 3], f32)
            nc.scalar.dma_start(out=w2_t, in_=w2_d)
            b2_t = head.tile([3, 1], f32)
            nc.scalar.dma_start(out=b2_t, in_=b2_d)
            idt = singles.tile([BPC, BPC], f32)
            make_identity(nc, idt)

            # --- bert row chunks on the SP ring, grouped ---
            bt = singles.tile([128, totch * H], bdt)
            c0 = 0
            for g in groups:
                nc.sync.dma_start(out=bt[:, c0 * H:(c0 + g) * H],
                                  in_=bert_d[:, c0 * H:(c0 + g) * H])
                c0 += g

            # --- W1 on the ACT ring, after consts (needed only by mm2) ---
            w1_t = singles.tile([128, KC, HID], wdt)
            for i in range(4):
                nc.scalar.dma_start(out=w1_t[:, 6 * i:6 * (i + 1), :],
                                    in_=w1_d[:, 6 * i:6 * (i + 1), :])

            # xT[p, hc, col]: col = e*BPC + slot for e in {A, B, pron}
            xT_t = singles.tile([128, HC, 3 * BPC], wdt)

            # --- pron embedding: fp32 rows, transposed via PE ---
            for hc in range(HC):
                pxp = psum_p.tile([128, BPC], f32, tag="pxp")
                nc.tensor.transpose(pxp, pron_t[:, hc * 128:(hc + 1) * 128],
                                    idt)
                nc.vector.tensor_copy(
                    xT_t[:, hc, 2 * BPC:3 * BPC], pxp)

            # --- mm1: span sums for all slots at once ---
            # The mask column encodes (span, slot), so chunks may mix
            # batches freely. PSUM accumulation groups must be closed
            # before the next opens (concurrent open groups in a bank
            # corrupt each other), so accumulate per DMA group in PSUM
            # and flush to an SBUF accumulator.
            xacc = singles.tile([128, HC, NMC], f32)
            c0 = 0
            for gi, g in enumerate(groups):
                for hc in range(HC):
                    pxg = psum_x.tile([128, NMC], f32, tag="px")
                    for j in range(g):
                        sc = c0 + j
                        nc.tensor.matmul(
                            pxg,
                            bt[:, sc * H + hc * 128:sc * H + (hc + 1) * 128],
                            mask_t[:, sc, :],
                            start=(j == 0),
                            stop=(j == g - 1),
                        )
                    if gi == 0:
                        nc.vector.tensor_copy(xacc[:, hc, :], pxg)
                    else:
                        nc.vector.tensor_add(xacc[:, hc, :],
                                             xacc[:, hc, :], pxg)
                c0 += g
            # fp32 scale by 1/span_len on the SBUF->SBUF copy
            for hc in range(HC):
                nc.vector.tensor_mul(xT_t[:, hc, 0:NMC], xacc[:, hc, :],
                                     sfac_t)

            # --- mm2: hT[q] = sum_kc W1sub.T @ xT chunk (24 k-chunks) ---
            phT = psum_h.tile([128, NQ, BPC], f32)
            for q in range(NQ):
                for kc in range(KC):
                    e, hc = kc // HC, kc % HC
                    nc.tensor.matmul(
                        phT[:, q, :],
                        w1_t[:, kc, q * 128:(q + 1) * 128],
                        xT_t[:, hc, e * BPC:(e + 1) * BPC],
                        start=(kc == 0),
                        stop=(kc == KC - 1),
                    )

            # --- BN bias + LeakyReLU + mm3, per hid quarter ---
            ot_ps = psum_h.tile([3, BPC], f32, tag="oT")
            for q in range(NQ):
                t_t = head.tile([128, BPC], f32, tag="t_t")
                nc.vector.tensor_scalar_add(t_t, phT[:, q, :],
                                            bnb_t[:, q:q + 1])
                y_t = head.tile([128, BPC], f32, tag="y_t")
                # y = max(0.01 * t, t)
                nc.vector.scalar_tensor_tensor(
                    y_t, t_t, 0.01, t_t,
                    op0=mybir.AluOpType.mult, op1=mybir.AluOpType.max)
                nc.tensor.matmul(ot_ps, w2_t[:, q, :], y_t,
                                 start=(q == 0), stop=(q == NQ - 1))

            o_t = head.tile([3, BPC], f32)
            nc.vector.tensor_scalar_add(o_t, ot_ps, b2_t)
            nc.sync.dma_start(out=out_d, in_=o_t)

    nc.compile()
    return nc


def _prep_core_inputs(bert8, bert_f32, offsets, batch_idx, totch, np8):
    """Build the per-core input map for the given batch indices."""
    NMC = 2 * BPC
    bertp = np.zeros((totch, 128, H), dtype=bert8.dtype)
    sfac = np.ones((NMC,), dtype=np.float32)
    pron = np.empty((BPC, H), dtype=np.float32)
    flat_bert = bertp.reshape(totch * 128, H)
    flat_mask = np.zeros((totch * 128, NMC), dtype=bert8.dtype)
    row = 0
    for slot, gb in enumerate(batch_idx):
        a0, a1, b0, b1_, p = (int(v) for v in offsets[gb])
        rows = np.union1d(np.arange(a0, a1 + 1), np.arange(b0, b1_ + 1))
        n = rows.shape[0]
        flat_bert[row:row + n] = bert8[gb, rows]
        flat_mask[row:row + n, 0 * BPC + slot] = \
            ((rows >= a0) & (rows <= a1)).astype(np.float32).astype(np8)
        flat_mask[row:row + n, 1 * BPC + slot] = \
            ((rows >= b0) & (rows <= b1_)).astype(np.float32).astype(np8)
        sfac[0 * BPC + slot] = 1.0 / (a1 - a0 + 1)
        sfac[1 * BPC + slot] = 1.0 / (b1_ - b0 + 1)
        pron[slot] = bert_f32[gb, p]
        row += n
    # partition-major layout: each SBUF partition line is contiguous DRAM
    maskp = np.ascontiguousarray(flat_mask.reshape(totch, 128, NMC)
                                 .transpose(1, 0, 2))
    return {
        "bertp": np.ascontiguousarray(
            bertp.transpose(1, 0, 2).reshape(128, totch * H)),
        "maskp": maskp,
        "sfac": np.broadcast_to(sfac, (128, NMC)).copy(),
        "pron": pron,
    }


def kernel(bert_outputs, offsets, W1, b1, gamma, beta, running_mean,
           running_var, W2, b2):
    import ml_dtypes

    np8 = ml_dtypes.float8_e4m3 if BERT_FP8 else ml_dtypes.bfloat16

    bert_f32 = np.ascontiguousarray(np.asarray(bert_outputs, dtype=np.float32))
    bert8 = bert_f32.astype(np8)
    offs = np.asarray(offsets).astype(np.int64)
    W1 = np.asarray(W1, dtype=np.float32)
    b1 = np.asarray(b1, dtype=np.float32)
    gamma = np.asarray(gamma, dtype=np.float32)
    beta = np.asarray(beta, dtype=np.float32)
    rm = np.asarray(running_mean, dtype=np.float32)
    rv = np.asarray(running_var, dtype=np.float32)
    W2 = np.asarray(W2, dtype=np.float32)
    b2 = np.asarray(b2, dtype=np.float32)

    # Fold BN eval-mode stats: bn(xW1 + b1) = x(W1*s) + ((b1 - mean)*s + beta)
    s = gamma / np.sqrt(rv + EPS)
    bias = (b1 - rm) * s + beta
    W1s = W1 * s[None, :]
    if W1_BF16:
        W1s = W1s.astype(ml_dtypes.bfloat16)
    w1P = np.ascontiguousarray(
        W1s.reshape(KC, 128, HID).transpose(1, 0, 2))
    bnbP = np.ascontiguousarray(bias.reshape(NQ, 128).T)
    w2P = np.ascontiguousarray(W2.reshape(NQ, 128, 3).transpose(1, 0, 2))
    b2c = np.ascontiguousarray(b2.reshape(3, 1))

    # Exact union rows per batch; balance total rows across cores (LPT,
    # exactly BPC batches per core).
    nrows = np.empty(B, dtype=np.int64)
    for gb in range(B):
        a0, a1, b0, b1_, _ = (int(v) for v in offs[gb])
        lo, hi = min(a0, b0), max(a1, b1_)
        span = hi - lo + 1
        # union size without materializing: overlap or disjoint
        if b0 <= a1 and a0 <= b1_:
            nrows[gb] = span
        else:
            nrows[gb] = (a1 - a0 + 1) + (b1_ - b0 + 1)
    order = np.argsort(-nrows, kind="stable")
    loads = np.zeros(NCORES, dtype=np.int64)
    counts = np.zeros(NCORES, dtype=np.int64)
    asg = [[] for _ in range(NCORES)]
    for gb in order:
        open_cores = np.flatnonzero(counts < BPC)
        c = open_cores[np.argmin(loads[open_cores])]
        asg[c].append(int(gb))
        loads[c] += nrows[gb]
        counts[c] += 1
    totch = int((loads.max() + 127) // 128)

    if totch not in _PROGRAM_CACHE:
        _PROGRAM_CACHE[totch] = _build_program(totch)
    nc = _PROGRAM_CACHE[totch]

    shared = {"w1P": w1P, "bnbP": bnbP, "w2P": w2P, "b2c": b2c}
    in_maps = []
    for c in range(NCORES):
        m = _prep_core_inputs(bert8, bert_f32, offs, asg[c], totch, np8)
        m.update(shared)
        in_maps.append(m)

    from concourse import bass_utils
    kwargs = {}
    if TRACE:
        kwargs = {"trace": True, "trace_cores": list(range(NCORES))}
    res = bass_utils.run_bass_kernel_spmd(nc, in_maps,
                                          core_ids=list(range(NCORES)),
                                          **kwargs)
    global LAST_RESULT
    LAST_RESULT = res

    out = np.empty((B, 3), dtype=np.float32)
    for c in range(NCORES):
        out[asg[c]] = res.results[c]["out"].T
    return out


# revision 10
# speedup vs baseline: 1.5335x; 1.1985x over previous
"""CorefHead Trainium2 kernel.

Reference computation (B=64, S=512, H=1024, HID=512):
  emb_a = span_mean(bert, offsets[:,0:2])   # [B,H]
  emb_b = span_mean(bert, offsets[:,2:4])   # [B,H]
  emb_p = bert[b, offsets[:,4]]             # [B,H]
  x = concat([emb_a, emb_b, emb_p], -1)     # [B,3H]
  h = leaky_relu(batchnorm_eval(x @ W1 + b1), 0.01)
  out = h @ W2 + b2                         # [B,3]

Strategy: pure data parallel, batch sharded 8 ways (8 batches/core),
DMA-volume minimized:
  - Host ships only the exact union rows (span A + span B) per batch,
    packed back-to-back across the core's 8 batches into 128-row chunks
    (chunks may cross batch boundaries). Rows are fp8-e4m3: span means
    average ~170 rows and the pron row dominates the final signal, so
    fp8 noise on span rows stays ~0.5% at the output. The pron rows ship
    separately in fp32 and are transposed on the PE.
  - mm1 (PE): per h-chunk a single PSUM tile [128, 16] accumulates
    bert_chunk.T @ mask_chunk over ALL row chunks; mask has one column
    per (span, slot) so batch identity lives in the mask column.
  - mm2 (PE, swapped operands): phT[q] += W1sub[128k, 128hid].T @
    xT[128k, 8] over 24 k-chunks -> h transposed [512, 8] directly (no
    on-device transpose of h), streaming only 8 columns per matmul.
  - BN+LeakyReLU (DVE) on hT tiles; mm3 (PE): out[3, 8] += w2q.T @ yq.
  - DMA: bert rides the SP ring in ~0.5 MB groups (first group small to
    prime the mm1 pipeline); consts + W1 ride the ACT ring; W1 is only
    needed by mm2 at the end so bert is never stuck behind it.
Host gathers per-core [3, 8] outputs and undoes the batch permutation.
"""

import numpy as np

B, S, H = 64, 512, 1024
HID = 512
EPS = 1e-5
NCORES = 8
BPC = B // NCORES  # batches per core
KC = 3 * H // 128  # 24 contraction chunks for mm2
HC = H // 128      # 8 h-chunks per embedding
NQ = HID // 128    # 4 hid quarters

# bert span rows + masks in fp8-e4m3 (halves DMA vs bf16); pron fp32.
BERT_FP8 = True
# Spans shorter than this get a second pass of fp8 residual rows
# (v - fp8(v), same mask column): short spans don't average away fp8
# noise, and two fp8 levels beat bf16 precision for ~5% extra rows.
LTHR = 64
# W1 (and the mm2 xT operand) in bf16.
W1_BF16 = True

# Test-harness hooks (harness calls kernel() with TRACE=False default).
TRACE = False
LAST_RESULT = None

_PROGRAM_CACHE: dict = {}


def _bert_groups(totch: int):
    """Chunk-group sizes for the bert DMA: small first group to prime
    the mm1 pipeline, then ~4-chunk (512 KB fp8) transfers."""
    if totch <= 2:
        return [totch]
    groups = [2]
    left = totch - 2
    while left > 0:
        g = min(4, left)
        groups.append(g)
        left -= g
    return groups


def _build_program(totch: int):
    import concourse.bacc as bacc
    import concourse.tile as tile
    import concourse.mybir as mybir
    from concourse.bass import MemorySpace
    from concourse.masks import make_identity

    f32 = mybir.dt.float32
    bdt = mybir.dt.float8e4 if BERT_FP8 else mybir.dt.bfloat16
    wdt = mybir.dt.bfloat16 if W1_BF16 else f32

    nc = bacc.Bacc("TRN2", target_bir_lowering=False, debug=False,
                   num_devices=NCORES)

    NMC = 2 * BPC  # mask columns: (span e, slot) -> e*BPC + slot

    bert_d = nc.dram_tensor("bertp", [128, totch * H], bdt,
                            kind="ExternalInput").ap()
    mask_d = nc.dram_tensor("maskp", [128, totch, NMC], bdt,
                            kind="ExternalInput").ap()
    sfac_d = nc.dram_tensor("sfac", [128, NMC], f32, kind="ExternalInput").ap()
    pron_d = nc.dram_tensor("pron", [BPC, H], f32, kind="ExternalInput").ap()
    w1_d = nc.dram_tensor("w1P", [128, KC, HID], wdt, kind="ExternalInput").ap()
    bnb_d = nc.dram_tensor("bnbP", [128, NQ], f32, kind="ExternalInput").ap()
    w2_d = nc.dram_tensor("w2P", [128, NQ, 3], f32, kind="ExternalInput").ap()
    b2_d = nc.dram_tensor("b2c", [3, 1], f32, kind="ExternalInput").ap()
    out_d = nc.dram_tensor("out", [3, BPC], f32, kind="ExternalOutput").ap()

    groups = _bert_groups(totch)

    with tile.TileContext(nc) as tc:
        with (
            tc.tile_pool(name="singles", bufs=1) as singles,
            tc.tile_pool(name="head", bufs=1) as head,
            tc.tile_pool(name="psum_x", bufs=3, space=MemorySpace.PSUM) as psum_x,
            tc.tile_pool(name="psum_p", bufs=2, space=MemorySpace.PSUM) as psum_p,
            tc.tile_pool(name="psum_h", bufs=1, space=MemorySpace.PSUM) as psum_h,
        ):
            # --- consts on the ACT ring (tiny, land first) ---
            mask_t = singles.tile([128, totch, NMC], bdt)
            nc.scalar.dma_start(out=mask_t, in_=mask_d)
            sfac_t = singles.tile([128, NMC], f32)
            nc.scalar.dma_start(out=sfac_t, in_=sfac_d)
            pron_t = singles.tile([BPC, H], f32)
            nc.scalar.dma_start(out=pron_t, in_=pron_d)
            bnb_t = head.tile([128, NQ], f32)
            nc.scalar.dma_start(out=bnb_t, in_=bnb_d)
            w2_t = head.tile([128, NQ, 3], f32)
            nc.scalar.dma_start(out=w2_t, in_=w2_d)
            b2_t = head.tile([3, 1], f32)
            nc.scalar.dma_start(out=b2_t, in_=b2_d)
            idt = singles.tile([BPC, BPC], f32)
            make_identity(nc, idt)

            # --- bert row chunks on the SP ring, grouped ---
            bt = singles.tile([128, totch * H], bdt)
            c0 = 0
            for g in groups:
                nc.sync.dma_start(out=bt[:, c0 * H:(c0 + g) * H],
                                  in_=bert_d[:, c0 * H:(c0 + g) * H])
                c0 += g

            # --- W1 on the ACT ring, after consts (needed only by mm2) ---
            w1_t = singles.tile([128, KC, HID], wdt)
            for i in range(4):
                nc.scalar.dma_start(out=w1_t[:, 6 * i:6 * (i + 1), :],
                                    in_=w1_d[:, 6 * i:6 * (i + 1), :])

            # xT[p, hc, col]: col = e*BPC + slot for e in {A, B, pron}
            xT_t = singles.tile([128, HC, 3 * BPC], wdt)

            # --- pron embedding: fp32 rows, transposed via PE ---
            for hc in range(HC):
                pxp = psum_p.tile([128, BPC], f32, tag="pxp")
                nc.tensor.transpose(pxp, pron_t[:, hc * 128:(hc + 1) * 128],
                                    idt)
                nc.vector.tensor_copy(
                    xT_t[:, hc, 2 * BPC:3 * BPC], pxp)

            # --- mm1: span sums for all slots at once ---
            # The mask column encodes (span, slot), so chunks may mix
            # batches freely. PSUM accumulation groups must be closed
            # before the next opens (concurrent open groups in a bank
            # corrupt each other), so accumulate per DMA group in PSUM
            # and flush to an SBUF accumulator.
            xacc = singles.tile([128, HC, NMC], f32)
            c0 = 0
            for gi, g in enumerate(groups):
                for hc in range(HC):
                    pxg = psum_x.tile([128, NMC], f32, tag="px")
                    for j in range(g):
                        sc = c0 + j
                        nc.tensor.matmul(
                            pxg,
                            bt[:, sc * H + hc * 128:sc * H + (hc + 1) * 128],
                            mask_t[:, sc, :],
                            start=(j == 0),
                            stop=(j == g - 1),
                        )
                    if gi == 0:
                        nc.vector.tensor_copy(xacc[:, hc, :], pxg)
                    else:
                        nc.vector.tensor_add(xacc[:, hc, :],
                                             xacc[:, hc, :], pxg)
                c0 += g
            # fp32 scale by 1/span_len on the SBUF->SBUF copy
            for hc in range(HC):
                nc.vector.tensor_mul(xT_t[:, hc, 0:NMC], xacc[:, hc, :],
                                     sfac_t)

            # --- mm2: hT[q] = sum_kc W1sub.T @ xT chunk (24 k-chunks) ---
            phT = psum_h.tile([128, NQ, BPC], f32)
            for q in range(NQ):
                for kc in range(KC):
                    e, hc = kc // HC, kc % HC
                    nc.tensor.matmul(
                        phT[:, q, :],
                        w1_t[:, kc, q * 128:(q + 1) * 128],
                        xT_t[:, hc, e * BPC:(e + 1) * BPC],
                        start=(kc == 0),
                        stop=(kc == KC - 1),
                    )

            # --- BN bias + LeakyReLU + mm3, per hid quarter ---
            ot_ps = psum_h.tile([3, BPC], f32, tag="oT")
            for q in range(NQ):
                t_t = head.tile([128, BPC], f32, tag="t_t")
                nc.vector.tensor_scalar_add(t_t, phT[:, q, :],
                                            bnb_t[:, q:q + 1])
                y_t = head.tile([128, BPC], f32, tag="y_t")
                # y = max(0.01 * t, t)
                nc.vector.scalar_tensor_tensor(
                    y_t, t_t, 0.01, t_t,
                    op0=mybir.AluOpType.mult, op1=mybir.AluOpType.max)
                nc.tensor.matmul(ot_ps, w2_t[:, q, :], y_t,
                                 start=(q == 0), stop=(q == NQ - 1))

            o_t = head.tile([3, BPC], f32)
            nc.vector.tensor_scalar_add(o_t, ot_ps, b2_t)
            nc.sync.dma_start(out=out_d, in_=o_t)

    nc.compile()
    return nc


def _prep_core_inputs(bert8, bert_f32, offsets, batch_idx, totch, np8):
    """Build the per-core input map for the given batch indices."""
    NMC = 2 * BPC
    bertp = np.zeros((totch, 128, H), dtype=bert8.dtype)
    sfac = np.ones((NMC,), dtype=np.float32)
    pron = np.empty((BPC, H), dtype=np.float32)
    flat_bert = bertp.reshape(totch * 128, H)
    flat_mask = np.zeros((totch * 128, NMC), dtype=bert8.dtype)
    one = np.ones((), dtype=np8)
    row = 0
    for slot, gb in enumerate(batch_idx):
        a0, a1, b0, b1_, p = (int(v) for v in offsets[gb])
        rows = np.union1d(np.arange(a0, a1 + 1), np.arange(b0, b1_ + 1))
        n = rows.shape[0]
        flat_bert[row:row + n] = bert8[gb, rows]
        flat_mask[row:row + n, 0 * BPC + slot] = \
            ((rows >= a0) & (rows <= a1)).astype(np.float32).astype(np8)
        flat_mask[row:row + n, 1 * BPC + slot] = \
            ((rows >= b0) & (rows <= b1_)).astype(np.float32).astype(np8)
        sfac[0 * BPC + slot] = 1.0 / (a1 - a0 + 1)
        sfac[1 * BPC + slot] = 1.0 / (b1_ - b0 + 1)
        pron[slot] = bert_f32[gb, p]
        row += n
        if BERT_FP8:
            for e, (s0, s1) in enumerate(((a0, a1), (b0, b1_))):
                L = s1 - s0 + 1
                if L >= LTHR:
                    continue
                res = (bert_f32[gb, s0:s1 + 1]
                       - bert8[gb, s0:s1 + 1].astype(np.float32))
                flat_bert[row:row + L] = res.astype(np8)
                flat_mask[row:row + L, e * BPC + slot] = one
                row += L
    # partition-major layout: each SBUF partition line is contiguous DRAM
    maskp = np.ascontiguousarray(flat_mask.reshape(totch, 128, NMC)
                                 .transpose(1, 0, 2))
    return {
        "bertp": np.ascontiguousarray(
            bertp.transpose(1, 0, 2).reshape(128, totch * H)),
        "maskp": maskp,
        "sfac": np.broadcast_to(sfac, (128, NMC)).copy(),
        "pron": pron,
    }


def kernel(bert_outputs, offsets, W1, b1, gamma, beta, running_mean,
           running_var, W2, b2):
    import ml_dtypes

    np8 = ml_dtypes.float8_e4m3 if BERT_FP8 else ml_dtypes.bfloat16

    bert_f32 = np.ascontiguousarray(np.asarray(bert_outputs, dtype=np.float32))
    bert8 = bert_f32.astype(np8)
    offs = np.asarray(offsets).astype(np.int64)
    W1 = np.asarray(W1, dtype=np.float32)
    b1 = np.asarray(b1, dtype=np.float32)
    gamma = np.asarray(gamma, dtype=np.float32)
    beta = np.asarray(beta, dtype=np.float32)
    rm = np.asarray(running_mean, dtype=np.float32)
    rv = np.asarray(running_var, dtype=np.float32)
    W2 = np.asarray(W2, dtype=np.float32)
    b2 = np.asarray(b2, dtype=np.float32)

    # Fold BN eval-mode stats: bn(xW1 + b1) = x(W1*s) + ((b1 - mean)*s + beta)
    s = gamma / np.sqrt(rv + EPS)
    bias = (b1 - rm) * s + beta
    W1s = W1 * s[None, :]
    if W1_BF16:
        W1s = W1s.astype(ml_dtypes.bfloat16)
    w1P = np.ascontiguousarray(
        W1s.reshape(KC, 128, HID).transpose(1, 0, 2))
    bnbP = np.ascontiguousarray(bias.reshape(NQ, 128).T)
    w2P = np.ascontiguousarray(W2.reshape(NQ, 128, 3).transpose(1, 0, 2))
    b2c = np.ascontiguousarray(b2.reshape(3, 1))

    # Exact union rows per batch; balance total rows across cores (LPT,
    # exactly BPC batches per core).
    nrows = np.empty(B, dtype=np.int64)
    for gb in range(B):
        a0, a1, b0, b1_, _ = (int(v) for v in offs[gb])
        # union size without materializing: overlap or disjoint
        if b0 <= a1 and a0 <= b1_:
            nrows[gb] = max(a1, b1_) - min(a0, b0) + 1
        else:
            nrows[gb] = (a1 - a0 + 1) + (b1_ - b0 + 1)
        if BERT_FP8:
            for s0, s1 in ((a0, a1), (b0, b1_)):
                if s1 - s0 + 1 < LTHR:
                    nrows[gb] += s1 - s0 + 1
    order = np.argsort(-nrows, kind="stable")
    loads = np.zeros(NCORES, dtype=np.int64)
    counts = np.zeros(NCORES, dtype=np.int64)
    asg = [[] for _ in range(NCORES)]
    for gb in order:
        open_cores = np.flatnonzero(counts < BPC)
        c = open_cores[np.argmin(loads[open_cores])]
        asg[c].append(int(gb))
        loads[c] += nrows[gb]
        counts[c] += 1
    totch = int((loads.max() + 127) // 128)

    if totch not in _PROGRAM_CACHE:
        _PROGRAM_CACHE[totch] = _build_program(totch)
    nc = _PROGRAM_CACHE[totch]

    shared = {"w1P": w1P, "bnbP": bnbP, "w2P": w2P, "b2c": b2c}
    in_maps = []
    for c in range(NCORES):
        m = _prep_core_inputs(bert8, bert_f32, offs, asg[c], totch, np8)
        m.update(shared)
        in_maps.append(m)

    from concourse import bass_utils
    kwargs = {}
    if TRACE:
        kwargs = {"trace": True, "trace_cores": list(range(NCORES))}
    res = bass_utils.run_bass_kernel_spmd(nc, in_maps,
                                          core_ids=list(range(NCORES)),
                                          **kwargs)
    global LAST_RESULT
    LAST_RESULT = res

    out = np.empty((B, 3), dtype=np.float32)
    for c in range(NCORES):
        out[asg[c]] = res.results[c]["out"].T
    return out
